# revision 31
# baseline (speedup 1.0000x reference)
"""AdaptiveEdgeWeightGNN (GCNConv with edge weights) on 8 Trainium2 NeuronCores.

V3: parity-split single-scale ELLPACK.  Table = x in bf16 natural pair
rows [x_2j | x_2j+1] (256B each).  Cells (dst,src) are split by src
parity into TWO independently-sorted ELLPACK grids: even-src cells
gather pair rows at byte offset 0 (x_src in the LO half), odd-src cells
gather at byte offset +128 (again x_src in the LO half).  Every cell
therefore needs ONE scale and no parity add.  The two grids have
independent node placements; since (aggE + aggO) @ W is linear, each
grid's @W output is computed separately on device and the two are
combined on the host during output assembly.

Device per call: dma_gather 256B rows -> DVE scale-mult (lo half only)
-> per-column identity matmul accumulate into PSUM windows -> per-bank
tail: cast+transpose -> @W (bf16) -> (+self/bias for E grid) -> DMA out.
"""
import os
import ml_dtypes
import numpy as np

import concourse.bacc as bacc
import concourse.bass as bass
import concourse.tile as tile
from concourse import mybir
from concourse.bass_utils import run_bass_kernel_spmd
from concourse.masks import make_identity

N_NODES = 50000
D = 64
N_CORES = 8
NPC = 6250            # real nodes per core per grid
PADN = 6272           # padded nodes per core (49 windows x 128)
N_WIN = PADN // 128   # 49
P = 128
NPAIR = 25000         # pair rows in the table (plus 1 zero pad row)
CC = int(os.environ.get("BASS_GNN_CC", "16"))      # max cols per gather call
SINGLE_PACKET = bool(int(os.environ.get("BASS_GNN_SP", "0")))
WPB = 8               # windows per bank-tail group

F32 = mybir.dt.float32
BF16 = mybir.dt.bfloat16
I16 = mybir.dt.int16
NSLOT = 16            # psum agg ring: windows live mod-16 (2 banks)


def _grid_place(dst, src16, nrm):
    """Build one parity grid: ELLPACK placement sorted by per-node count."""
    cnt = np.bincount(dst, minlength=N_NODES)
    order = np.argsort(-cnt, kind="stable")
    grank = np.empty(N_NODES, dtype=np.int64)
    grank[order] = np.arange(N_NODES)
    owner = grank % N_CORES
    lrank = grank // N_CORES

    csort = cnt[order]
    K = np.zeros(N_WIN, dtype=np.int64)
    for w in range(N_WIN):
        s = w * 128 * N_CORES
        K[w] = csort[s] if s < N_NODES else 0
    K = np.maximum(K, 1)
    off = np.zeros(N_WIN + 1, dtype=np.int64)
    off[1:] = np.cumsum(K)
    COLS = int(off[-1])

    gidx = np.zeros((N_CORES, P, COLS), dtype=np.int16)
    s2 = np.zeros((N_CORES, P, COLS), dtype=np.float32)

    own_c = owner[dst]
    lr = lrank[dst]
    wn = lr // P
    pp = lr - wn * P
    seg_start = np.searchsorted(dst, dst)   # dst sorted ascending
    j = np.arange(dst.size) - seg_start
    colpos = off[wn] + j
    assert (j < K[wn]).all()
    gidx[own_c, pp, colpos] = src16
    s2[own_c, pp, colpos] = nrm

    node_at_rank = np.full((N_CORES, PADN), -1, dtype=np.int64)
    for c in range(N_CORES):
        node_at_rank[c, :NPC] = order[c::N_CORES]
    return dict(COLS=COLS, off=off, gidx=gidx, s2=s2,
                node_at_rank=node_at_rank)


def _call_list(COLS, ramp=False):
    """Descending column chunks; the remainder chunk (cols 0..r-1) is
    processed last so the kernel tail is short.  With ramp=True the
    first few calls are small so early descriptor generation never
    blocks mid-call while the DMA drain path warms up."""
    calls = []
    c0 = COLS
    sizes = iter([4, 4, 4, 4, 8, 8] if ramp else [])
    while c0 > 0:
        nc = min(next(sizes, CC), c0)
        c0 -= nc
        calls.append((c0, nc))
    return calls


def _wrap_idx(gidx_call):
    """[P, nc] int16 -> wrapped [P, nc*8] layout for dma_gather."""
    nc = gidx_call.shape[1]
    flat = gidx_call.T.reshape(-1)                       # col-major (c,p)
    w16 = flat.reshape(nc * 8, 16).T                     # [16, nc*8]
    return np.tile(w16, (8, 1)).copy()                   # [128, nc*8]


def _preprocess(edge_index: np.ndarray, edge_weight: np.ndarray):
    row = np.asarray(edge_index[0], dtype=np.int64)
    col = np.asarray(edge_index[1], dtype=np.int64)
    ew = np.asarray(edge_weight, dtype=np.float64)

    # add self-loops (weight 1.0), merge parallel edges by (dst, src)
    loop = np.arange(N_NODES, dtype=np.int64)
    row = np.concatenate([row, loop])
    col = np.concatenate([col, loop])
    ew = np.concatenate([ew, np.ones(N_NODES)])
    key = col * N_NODES + row
    order0 = np.argsort(key, kind="stable")
    ks = key[order0]
    uniq = np.empty(ks.shape, dtype=bool)
    uniq[0] = True
    uniq[1:] = ks[1:] != ks[:-1]
    seg = np.cumsum(uniq) - 1
    ew_m = np.bincount(seg, weights=ew[order0])
    ku = ks[uniq]
    dst_m = ku // N_NODES
    src_m = ku % N_NODES

    deg = np.bincount(dst_m, weights=ew_m, minlength=N_NODES)
    dis = np.where(deg > 0, 1.0 / np.sqrt(deg), 0.0)
    norm_m = dis[src_m] * ew_m * dis[dst_m]

    selfm = src_m == dst_m
    selfcoef = np.zeros(N_NODES)
    selfcoef[dst_m[selfm]] = norm_m[selfm]

    dst_e = dst_m[~selfm]
    src_e = src_m[~selfm]
    nrm_e = norm_m[~selfm]

    par = (src_e & 1).astype(bool)
    grids = {}
    for gname, mask in (("E", ~par), ("O", par)):
        d = dst_e[mask]
        s16 = (src_e[mask] >> 1).astype(np.int16)
        grids[gname] = _grid_place(d, s16, nrm_e[mask].astype(np.float32))

    callsE = _call_list(grids["E"]["COLS"])
    callsO = _call_list(grids["O"]["COLS"])

    # concatenated per-call wrapped index stream + per-col scale stream
    gi_parts = [[] for _ in range(N_CORES)]
    for gname, calls in (("E", callsE), ("O", callsO)):
        g = grids[gname]
        for c in range(N_CORES):
            for c0, nc in calls:
                gi_parts[c].append(_wrap_idx(g["gidx"][c][:, c0:c0 + nc]))
    gi_w = np.stack([np.concatenate(p, axis=1) for p in gi_parts])
    s2_all = np.concatenate([grids["E"]["s2"], grids["O"]["s2"]], axis=2)

    return dict(grids=grids, callsE=callsE, callsO=callsO,
                gi_w=gi_w, s2_all=s2_all, selfcoef=selfcoef)


def _build_nc(offE, offO, callsE, callsO):
    COLSE = int(offE[-1])
    COLSO = int(offO[-1])
    TOT = COLSE + COLSO
    nc_ = bacc.Bacc("TRN2", target_bir_lowering=False, debug=False,
                    num_devices=N_CORES, num_swdge_queues=4)
    tab_in = nc_.dram_tensor("tab", [(NPAIR + 1) * 2 * D], BF16,
                             kind="ExternalInput")
    gi_in = nc_.dram_tensor("gidx", [P, TOT * 8], I16, kind="ExternalInput")
    s2_in = nc_.dram_tensor("s2", [P, TOT, 1], BF16, kind="ExternalInput")
    w_in = nc_.dram_tensor("W", [D, D], BF16, kind="ExternalInput")
    st_in = nc_.dram_tensor("selfterm", [PADN, D], BF16, kind="ExternalInput")
    outE_t = nc_.dram_tensor("outE", [PADN, D], F32, kind="ExternalOutput")
    outO_t = nc_.dram_tensor("outO", [PADN, D], F32, kind="ExternalOutput")

    apE = tab_in[:NPAIR * 2 * D].rearrange("(r e) -> r e", e=2 * D)
    apO = tab_in[D:D + NPAIR * 2 * D].rearrange("(r e) -> r e", e=2 * D)

    with tile.TileContext(nc_) as tc:
        with tc.tile_pool(name="const", bufs=1) as cp, \
             tc.tile_pool(name="work2", bufs=2) as wp2, \
             tc.tile_pool(name="gq", bufs=6) as gq, \
             tc.tile_pool(name="fq", bufs=6) as fq, \
             tc.tile_pool(name="pa", bufs=1, space="PSUM") as pa, \
             tc.tile_pool(name="ps", bufs=1, space="PSUM") as ps:

            # per-call gather-index tiles; only a few are prefetched up
            # front so their DMAs don't crowd the early gather drains
            allcalls = [("E", c0, nc) for c0, nc in callsE] + \
                       [("O", c0, nc) for c0, nc in callsO]
            goffs = np.concatenate([[0], np.cumsum(
                [nc for _, _, nc in allcalls])])
            gi_tiles = [cp.tile([P, nc * 8], I16, tag=f"gi{k}",
                                name=f"gi{k}")
                        for k, (_, _, nc) in enumerate(allcalls)]
            for k in range(len(allcalls)):
                nc_.sync.dma_start(
                    gi_tiles[k][:],
                    gi_in[:, int(goffs[k]) * 8:int(goffs[k + 1]) * 8])

            s2_t = cp.tile([P, TOT, 1], BF16, tag="s2")
            nc_.sync.dma_start(s2_t[:], s2_in[:])
            ident_f = cp.tile([P, P], F32, tag="idf")
            make_identity(nc_, ident_f[:])
            ident_b = cp.tile([P, P], BF16, tag="idb")
            nc_.vector.tensor_copy(ident_b[:], ident_f[:])
            w_sb = cp.tile([D, D], BF16, tag="w")
            nc_.sync.dma_start(w_sb[:], w_in[:])
            st_t = cp.tile([P, N_WIN, D], BF16, tag="st")
            nc_.sync.dma_start(
                st_t[:], st_in[:].rearrange("(w p) f -> p w f", p=P))

            agg = pa.tile([P, N_WIN * D], F32, tag="agg")

            def bank_tail(gname, off, b):
                w0 = b * WPB
                bw = min(WPB, N_WIN - w0)
                tmp = wp2.tile([P, WPB * D], BF16, tag="tmpagg")
                nc_.vector.tensor_copy(tmp[:, :bw * D],
                                       agg[:, w0 * D:(w0 + bw) * D])
                for s0 in range(0, bw, 4):
                    sw = min(4, bw - s0)
                    pt = ps.tile([D, 4 * P], BF16, tag="small")
                    for i in range(sw):
                        nc_.tensor.transpose(
                            out=pt[:, i * P:(i + 1) * P],
                            in_=tmp[:, (s0 + i) * D:(s0 + i + 1) * D],
                            identity=ident_b[:])
                    at = wp2.tile([D, 4 * P], BF16, tag="aggT")
                    nc_.vector.tensor_copy(at[:, :sw * P], pt[:, :sw * P])
                    for i in range(sw):
                        w = w0 + s0 + i
                        nc_.tensor.matmul(
                            out=agg[:, w * D:(w + 1) * D],
                            lhsT=at[:, i * P:(i + 1) * P],
                            rhs=w_sb[:], start=True, stop=True)
                outr = wp2.tile([P, WPB, D], F32, tag="outr")
                aggv = agg[:, w0 * D:(w0 + bw) * D].rearrange(
                    "p (w f) -> p w f", f=D)
                if gname == "E":
                    nc_.vector.tensor_tensor(
                        out=outr[:, :bw, :], in0=aggv,
                        in1=st_t[:, w0:w0 + bw, :],
                        op=mybir.AluOpType.add)
                else:
                    nc_.vector.tensor_copy(outr[:, :bw, :], aggv)
                out_t = outE_t if gname == "E" else outO_t
                nc_.sync.dma_start(
                    out_t[:].rearrange("(w p) f -> p w f", p=P)[:, w0:w0 + bw, :],
                    outr[:, :bw, :])

            k = 0
            for gname, off, calls, ap, s2base in (
                    ("E", offE, callsE, apE, 0),
                    ("O", offO, callsO, apO, COLSE)):
                first_col = {int(off[w + 1]) - 1: w for w in range(N_WIN)}
                last_col = {int(off[w]): w for w in range(N_WIN)}
                n_banks = (N_WIN + WPB - 1) // WPB
                next_bank = n_banks - 1
                for c0, nc in calls:
                    gt = gi_tiles[k]
                    g = gq.tile([P, CC, 2 * D], BF16, tag="g")
                    nc_.gpsimd.dma_gather(
                        out_ap=g[:, :nc, :], in_ap=ap,
                        idxs_ap=gt[:],
                        num_idxs=nc * P, num_idxs_reg=nc * P,
                        elem_size=2 * D, single_packet=SINGLE_PACKET,
                        queue_num=k % 4)
                    m2 = fq.tile([P, CC, D], BF16, tag="m2")
                    nc_.vector.tensor_tensor(
                        out=m2[:, :nc, :],
                        in0=g[:, :nc, 0:D],
                        in1=s2_t[:, s2base + c0:s2base + c0 + nc, :]
                            .to_broadcast([P, nc, D]),
                        op=mybir.AluOpType.mult)
                    for cl in reversed(range(nc)):
                        colx = c0 + cl
                        w = int(np.searchsorted(off, colx, side="right")) - 1
                        nc_.tensor.matmul(out=agg[:, w * D:(w + 1) * D],
                                          lhsT=ident_b[:], rhs=m2[:, cl, :],
                                          start=(colx in first_col),
                                          stop=(colx in last_col))
                    k += 1
                    while next_bank >= 0:
                        if int(off[next_bank * WPB]) >= c0:
                            bank_tail(gname, off, next_bank)
                            next_bank -= 1
                        else:
                            break
                while next_bank >= 0:
                    bank_tail(gname, off, next_bank)
                    next_bank -= 1

    nc_.compile()
    return nc_


_CACHE: dict = {}


def kernel(x, W, bias, edge_weight, edge_index) -> np.ndarray:
    x = np.asarray(x, dtype=np.float32)
    W = np.asarray(W, dtype=np.float32)
    bias = np.asarray(bias, dtype=np.float32)
    edge_weight = np.asarray(edge_weight, dtype=np.float32)
    edge_index = np.asarray(edge_index)

    pre = _preprocess(edge_index, edge_weight)
    gE, gO = pre["grids"]["E"], pre["grids"]["O"]

    ck = (tuple(gE["off"].tolist()), tuple(gO["off"].tolist()), CC,
          SINGLE_PACKET)
    if ck not in _CACHE:
        _CACHE[ck] = _build_nc(gE["off"], gO["off"],
                               pre["callsE"], pre["callsO"])
    nc_ = _CACHE[ck]

    tab = np.zeros(((NPAIR + 1) * 2 * D,), dtype=ml_dtypes.bfloat16)
    tab[:N_NODES * D] = x.reshape(-1).astype(ml_dtypes.bfloat16)
    self_full = (x * pre["selfcoef"][:, None].astype(np.float32)) @ W \
        + bias[None, :]

    in_maps = []
    for c in range(N_CORES):
        st = np.zeros((PADN, D), dtype=np.float32)
        nr = gE["node_at_rank"][c]
        real = nr >= 0
        st[real] = self_full[nr[real]]
        in_maps.append({
            "tab": tab,
            "gidx": np.ascontiguousarray(pre["gi_w"][c]),
            "s2": np.ascontiguousarray(pre["s2_all"][c])[..., None].astype(
                ml_dtypes.bfloat16),
            "W": W.astype(ml_dtypes.bfloat16),
            "selfterm": st.astype(ml_dtypes.bfloat16),
        })

    trace = bool(int(os.environ.get("BASS_GNN_TRACE", "0")))
    res = run_bass_kernel_spmd(nc_, in_maps, core_ids=list(range(N_CORES)),
                               trace=trace)
    if trace:
        kernel.last_exec_ns = res.exec_time_ns
        kernel.last_trace = (res.instructions_and_trace[1]
                             if res.instructions_and_trace else None)

    out = np.zeros((N_NODES, D), dtype=np.float32)
    for c in range(N_CORES):
        nr = gE["node_at_rank"][c]
        real = nr >= 0
        out[nr[real]] = res.results[c]["outE"][real]
    for c in range(N_CORES):
        nr = gO["node_at_rank"][c]
        real = nr >= 0
        out[nr[real]] += res.results[c]["outO"][real]
    return out
